# revision 1
# baseline (speedup 1.0000x reference)
"""DiffAttnV2-like fused kernel for Trainium2 (8 NeuronCores).

Sharding: core = 4*b + g  (b = batch 0..1, g = head-group 0..3, 4 heads each).
Each core computes its 4 output heads' attention and a partial out = y_g @ Wo_g;
host sums the 4 partials per batch.

Per-core dataflow (float32r matmuls - full PE rate, ~1.5e-4 rel rounding):
  4 phases over t-columns (512 each):
    projections into transposed layouts (qT/kT [d,t]; v natural [t,d]; lamT)
    causal attention in sT=[tk,tq] layout; ACT exp evacuates PSUM;
    denominator via ones-column matmul; normalize/combine via K=1 broadcast
    matmuls; partial output projection streamed per 512-col group.
"""
import sys
sys.path.insert(0, "/opt/trn_rl_repo")
from contextlib import ExitStack

import numpy as np

from concourse import bacc, mybir, tile
from concourse.bass_utils import run_bass_kernel_spmd

B, T, D, H = 2, 2048, 2048, 16
HPC = 4               # heads per core
NC = 8                # cores
NDC = D // 128        # 16 contraction chunks
NPH = 4               # t-phases
PT = T // NPH         # 512 t-cols per phase
SCALE = 1.0 / float(np.sqrt(D // H))

f32 = mybir.dt.float32
f32r = mybir.dt.float32r
EXP = mybir.ActivationFunctionType.Exp
SIG = mybir.ActivationFunctionType.Sigmoid

_CACHE = {}


def _build():
    nc = bacc.Bacc("TRN2", target_bir_lowering=False, debug=False)
    xTp = nc.dram_tensor("xTp", [NPH, 128, NDC, PT], f32r, kind="ExternalInput").ap()
    wqp = nc.dram_tensor("wqp", [8, 128, NDC, 128], f32r, kind="ExternalInput").ap()
    wkp = nc.dram_tensor("wkp", [HPC, 128, NDC, 128], f32r, kind="ExternalInput").ap()
    wvp = nc.dram_tensor("wvp", [2, 128, NDC, 256], f32r, kind="ExternalInput").ap()
    wlamp = nc.dram_tensor("wlamp", [128, NDC, HPC], f32r, kind="ExternalInput").ap()
    wop = nc.dram_tensor("wop", [4, 128, HPC, 512], f32r, kind="ExternalInput").ap()
    mstrip = nc.dram_tensor("mstrip", [128, 896], f32r, kind="ExternalInput").ap()
    selin = nc.dram_tensor("selin", [HPC, 512], f32r, kind="ExternalInput").ap()
    idin = nc.dram_tensor("idin", [128, 128], f32r, kind="ExternalInput").ap()
    onesin = nc.dram_tensor("onesin", [128, 4], f32r, kind="ExternalInput").ap()
    sel2in = nc.dram_tensor("sel2in", [2, 256], f32r, kind="ExternalInput").ap()
    out = nc.dram_tensor("out", [T, D], f32, kind="ExternalOutput").ap()

    with tile.TileContext(nc) as tc, ExitStack() as ctx:
        ctx.enter_context(nc.allow_low_precision(reason="fp32r matmul pipeline"))
        persist = ctx.enter_context(tc.tile_pool(name="persist", bufs=1))
        xpool = ctx.enter_context(tc.tile_pool(name="xpool", bufs=1))
        qpool = ctx.enter_context(tc.tile_pool(name="qpool", bufs=1))
        wpool = ctx.enter_context(tc.tile_pool(name="wpool", bufs=2))
        epool = ctx.enter_context(tc.tile_pool(name="epool", bufs=2))
        cpool = ctx.enter_context(tc.tile_pool(name="cpool", bufs=1))
        opool = ctx.enter_context(tc.tile_pool(name="opool", bufs=2))
        # PSUM banks: s4 (4x1) + acc (1x2) + den (1x1) + tr (1x1) = 8
        pps = ctx.enter_context(tc.tile_pool(name="pps", bufs=1, space="PSUM"))
        ppacc = ctx.enter_context(tc.tile_pool(name="ppacc", bufs=2, space="PSUM"))
        ppden = ctx.enter_context(tc.tile_pool(name="ppden", bufs=1, space="PSUM"))
        pptr = ctx.enter_context(tc.tile_pool(name="pptr", bufs=1, space="PSUM"))

        kT = persist.tile([128, HPC, T], f32r)          # 32KB
        vn = persist.tile([128, 2, NDC, 2, 128], f32r)  # 32KB [tk,(pair,tkc,j),d]
        ms = persist.tile([128, 896], f32r)             # -1e30/0 causal strip
        nc.sync.dma_start(out=ms[:], in_=mstrip[:])
        sel = persist.tile([HPC, HPC, 128], f32r)       # head-row selectors
        nc.sync.dma_start(out=sel.rearrange("p a b -> p (a b)"), in_=selin[:])
        iden = persist.tile([128, 128], f32r)           # identity for mask-add
        nc.sync.dma_start(out=iden[:], in_=idin[:])
        ones2 = persist.tile([128, 2, 2], f32r)     # den-row selector stationaries
        nc.sync.dma_start(out=ones2.rearrange("p a b -> p (a b)"), in_=onesin[:])
        sel2 = persist.tile([2, 2, 128], f32r)      # den-row broadcast selectors
        nc.sync.dma_start(out=sel2.rearrange("p a b -> p (a b)"), in_=sel2in[:])
        ones_row_f = persist.tile([1, 128], f32)
        nc.vector.memset(ones_row_f[:], 1.0)
        ones_row = persist.tile([1, 128], f32r)
        nc.vector.tensor_copy(ones_row[:], ones_row_f[:])

        for ph in range(NPH):
            t0 = PT * ph
            # ---- x^T slice for this phase ----
            xTh = xpool.tile([128, NDC, PT], f32r, name=f"xTh{ph}", tag="xTh")
            nc.sync.dma_start(out=xTh[:], in_=xTp[ph])

            # ---- q projections ----
            qTh = qpool.tile([128, 8, PT], f32r, name=f"qTh{ph}", tag="qTh")
            for qh in range(8):
                wt = wpool.tile([128, NDC, 128], f32r, name=f"wq{ph}_{qh}", tag="wq")
                nc.sync.dma_start(out=wt[:], in_=wqp[qh])
                ps = pptr.tile([128, PT], f32, name=f"psq{ph}_{qh}", tag="tr")
                for dc in range(NDC):
                    nc.tensor.matmul(ps[:], wt[:, dc], xTh[:, dc],
                                     start=(dc == 0), stop=(dc == NDC - 1))
                nc.vector.tensor_copy(qTh[:, qh], ps[:])

            # ---- k projections ----
            for kh in range(HPC):
                wt = wpool.tile([128, NDC, 128], f32r, name=f"wk{ph}_{kh}", tag="wq")
                nc.sync.dma_start(out=wt[:], in_=wkp[kh])
                ps = pptr.tile([128, PT], f32, name=f"psk{ph}_{kh}", tag="tr")
                for dc in range(NDC):
                    nc.tensor.matmul(ps[:], wt[:, dc], xTh[:, dc],
                                     start=(dc == 0), stop=(dc == NDC - 1))
                nc.vector.tensor_copy(kT[:, kh, t0:t0 + PT], ps[:])

            # ---- v projections (natural [tk, d]) ----
            for pair in range(2):
                wt = wpool.tile([128, NDC, 256], f32r, name=f"wv{ph}_{pair}",
                                tag="wv", bufs=1)
                nc.sync.dma_start(out=wt[:], in_=wvp[pair])
                for tsub in range(4):
                    tkc = 4 * ph + tsub
                    ps = pptr.tile([128, 256], f32, name=f"psv{ph}_{pair}_{tsub}",
                                   tag="tr")
                    for dc in range(NDC):
                        nc.tensor.matmul(
                            ps[:], xTh[:, dc, 128 * tsub:128 * (tsub + 1)],
                            wt[:, dc], start=(dc == 0), stop=(dc == NDC - 1))
                    nc.vector.tensor_copy(
                        vn[:, pair, tkc].rearrange("p a b -> p (a b)"), ps[:])

            # ---- lam projection + sigmoid ----
            wlt = wpool.tile([128, NDC, HPC], f32r, name=f"wl{ph}", tag="wl")
            nc.sync.dma_start(out=wlt[:], in_=wlamp[:])
            psl = pptr.tile([HPC, PT], f32, name=f"psl{ph}", tag="tr")
            for dc in range(NDC):
                nc.tensor.matmul(psl[:], wlt[:, dc], xTh[:, dc],
                                 start=(dc == 0), stop=(dc == NDC - 1))
            lamS = cpool.tile([HPC, PT], f32r, name=f"lam{ph}", tag="lam", bufs=1)
            nc.scalar.activation(lamS[:], psl[:], SIG)

            # ---- attention: 4 head-pairs, j0/j1 interleaved through one s-tile ----
            ntk = 4 * (ph + 1)
            yh = qpool.tile([128, HPC, PT], f32r, name=f"yh{ph}", tag="yh")
            pending_combine = None
            for hl in range(HPC):
                meta = []
                for j, qh in ((0, hl), (1, 4 + hl)):
                    khl = (hl // 2) if j == 0 else (2 + hl // 2)
                    meta.append((qh, khl, khl // 2, khl % 2))
                ps_y = [ppacc.tile([128, PT], f32, name=f"psy{ph}_{hl}_{j}",
                                   tag="acc") for j in range(2)]
                ps_den = ppden.tile([2, PT], f32, name=f"psd{ph}_{hl}", tag="den")

                def consume(bt, exs):
                    for j in range(2):
                        _, _, pair, pj = meta[j]
                        for cc in range(2):
                            tkc = 2 * bt + cc
                            exc = exs[j][:, cc]
                            nc.tensor.matmul(ps_den[0:2, :], ones2[:, j], exc,
                                             start=(j == 0 and tkc == 0),
                                             stop=(j == 1 and tkc == ntk - 1))
                            nc.tensor.matmul(ps_y[j][:], vn[:, pair, tkc, pj], exc,
                                             start=(tkc == 0), stop=(tkc == ntk - 1))

                prev = None
                for bt in range(ntk // 2):
                    if bt == 1 and pending_combine is not None:
                        pending_combine()
                        pending_combine = None
                    ps_s = pps.tile([128, 4, PT], f32, name=f"pss{ph}_{hl}_{bt}",
                                    tag="s4")
                    for j in range(2):
                        qh, khl = meta[j][0], meta[j][1]
                        for cc in range(2):
                            tkc = 2 * bt + cc
                            c = 2 * j + cc
                            o = 128 * tkc - t0
                            diag = o >= 0
                            nc.tensor.matmul(
                                ps_s[:, c],
                                kT[:, khl, 128 * tkc:128 * (tkc + 1)],
                                qTh[:, qh], start=True, stop=not diag)
                            if diag:
                                nc.tensor.matmul(ps_s[:, c], iden[:],
                                                 ms[:, 384 - o:896 - o],
                                                 start=False, stop=True)
                    exA = epool.tile([128, 2, PT], f32r,
                                     name=f"exA{ph}_{hl}_{bt}", tag="ex", bufs=3)
                    nc.scalar.activation(exA[:], ps_s[:, 0:2], EXP, scale=SCALE)
                    exB = epool.tile([128, 2, PT], f32r,
                                     name=f"exB{ph}_{hl}_{bt}", tag="ex", bufs=3)
                    nc.scalar.activation(exB[:], ps_s[:, 2:4], EXP, scale=SCALE)
                    if prev is not None:
                        consume(*prev)
                    prev = (bt, (exA, exB))
                consume(*prev)

                # combine y_h = y0*r0 - lam_h*(r1*y1); deferred to overlap
                rd_f = cpool.tile([2, PT], f32, name=f"rdf{ph}_{hl}", tag="rdf")
                nc.vector.reciprocal_approx_fast(rd_f[:], ps_den[0:2, :])
                rden2 = cpool.tile([2, PT], f32r, name=f"rden{ph}_{hl}", tag="rden")
                nc.scalar.copy(rden2[:], rd_f[:])

                def _combine(hl=hl, ps_y=ps_y, rden2=rden2):
                    t12 = []
                    for j in range(2):
                        ps_b = pptr.tile([128, PT], f32, name=f"psb{ph}_{hl}_{j}",
                                         tag="tr")
                        nc.tensor.matmul(ps_b[:], sel2[:, j], rden2[0:2, :],
                                         start=True, stop=True)
                        rB = cpool.tile([128, PT], f32, name=f"rB{ph}_{hl}_{j}",
                                        tag=f"rB{j}")
                        nc.vector.tensor_copy(rB[:], ps_b[:])
                        tj = cpool.tile([128, PT], f32, name=f"t{j}_{ph}_{hl}",
                                        tag=f"t{j}")
                        nc.vector.tensor_mul(tj[:], ps_y[j][:], rB[:])
                        t12.append(tj)
                    ps_lam = pptr.tile([128, PT], f32, name=f"pslam{ph}_{hl}",
                                       tag="tr")
                    nc.tensor.matmul(ps_lam[:], sel[:, hl], lamS[:],
                                     start=True, stop=True)
                    nc.vector.tensor_mul(t12[1][:], t12[1][:], ps_lam[:])
                    nc.vector.tensor_sub(yh[:, hl], t12[0][:], t12[1][:])

                if hl < HPC - 1 and ntk >= 4:
                    pending_combine = _combine
                else:
                    _combine()

            # ---- Wo partial ----
            for dout in range(4):
                wo4 = wpool.tile([128, HPC, 512], f32r, name=f"wo{ph}_{dout}",
                                 tag="wo4", bufs=2)
                nc.sync.dma_start(out=wo4[:], in_=wop[dout])
                for tsub in range(4):
                    alt = (dout * 4 + tsub) % 3
                    opl = ppacc if alt < 2 else ppden
                    ps_o = opl.tile([128, 512], f32, name=f"pso{ph}_{dout}_{tsub}",
                                    tag="acc" if alt < 2 else "den")
                    for hl in range(HPC):
                        nc.tensor.matmul(
                            ps_o[:], yh[:, hl, 128 * tsub:128 * (tsub + 1)],
                            wo4[:, hl], start=(hl == 0), stop=(hl == HPC - 1))
                    ob = opool.tile([128, 512], f32, name=f"ob{ph}_{dout}_{tsub}",
                                    tag="ob")
                    if (dout * 4 + tsub) % 2 == 0:
                        nc.vector.tensor_copy(ob[:], ps_o[:])
                    else:
                        nc.scalar.copy(ob[:], ps_o[:])
                    nc.sync.dma_start(
                        out=out[t0 + 128 * tsub:t0 + 128 * (tsub + 1),
                                512 * dout:512 * (dout + 1)],
                        in_=ob[:])
    nc.compile()
    return nc


def _get_nc():
    if "nc" not in _CACHE:
        _CACHE["nc"] = _build()
    return _CACHE["nc"]


def kernel(x, Wq1, Wq2, Wk, Wv, Wlam, Wo, **_ignored):
    x = np.ascontiguousarray(np.asarray(x, dtype=np.float32))
    Wq1 = np.asarray(Wq1, dtype=np.float32)
    Wq2 = np.asarray(Wq2, dtype=np.float32)
    Wk = np.asarray(Wk, dtype=np.float32)
    Wv = np.asarray(Wv, dtype=np.float32)
    Wlam = np.asarray(Wlam, dtype=np.float32)
    Wo = np.asarray(Wo, dtype=np.float32)

    cc = np.arange(896)[None, :]
    rr = np.arange(128)[:, None]
    mask = np.where(cc >= rr + 384, 0.0, -1e30).astype(np.float32)
    idv = np.eye(128, dtype=np.float32)
    ones2 = np.zeros((128, 2, 2), dtype=np.float32)
    ones2[:, 0, 0] = 1.0
    ones2[:, 1, 1] = 1.0
    ones2 = ones2.reshape(128, 4)
    sel2 = np.zeros((2, 2, 128), dtype=np.float32)
    sel2[0, 0, :] = 1.0
    sel2[1, 1, :] = 1.0
    sel2 = sel2.reshape(2, 256)
    selv = np.zeros((HPC, HPC, 128), dtype=np.float32)
    for i in range(HPC):
        selv[i, i, :] = 1.0
    selv = selv.reshape(HPC, 512)

    def chunk_cols(w):
        # [D, C] -> [C//128 heads? no: generic [D, C] -> [C/128? ] ] handled per-use
        return w

    xTs = []
    for b in range(B):
        xt = x[b].T                                   # [D, T]
        xTs.append(np.ascontiguousarray(
            xt.reshape(NDC, 128, NPH, PT).transpose(2, 1, 0, 3)))

    in_maps = []
    for core in range(NC):
        b, g = divmod(core, 4)
        kv_cols = np.r_[256 * g:256 * g + 256, 1024 + 256 * g:1024 + 256 * g + 256]
        wq_s = np.concatenate([Wq1[:, 512 * g:512 * (g + 1)],
                               Wq2[:, 512 * g:512 * (g + 1)]], axis=1)  # [D, 1024]
        wqp_v = np.ascontiguousarray(
            wq_s.reshape(NDC, 128, 8, 128).transpose(2, 1, 0, 3))
        wk_s = Wk[:, kv_cols]
        wkp_v = np.ascontiguousarray(
            wk_s.reshape(NDC, 128, HPC, 128).transpose(2, 1, 0, 3))
        wv_s = Wv[:, kv_cols]
        wvp_v = np.ascontiguousarray(
            wv_s.reshape(NDC, 128, 2, 256).transpose(2, 1, 0, 3))
        wlam_s = Wlam[:, 4 * g:4 * (g + 1)]
        wlamp_v = np.ascontiguousarray(
            wlam_s.reshape(NDC, 128, HPC).transpose(1, 0, 2))
        wo_s = Wo[512 * g:512 * (g + 1), :]
        wop_v = np.ascontiguousarray(
            wo_s.reshape(HPC, 128, 4, 512).transpose(2, 1, 0, 3))
        in_maps.append({
            "xTp": xTs[b],
            "wqp": wqp_v,
            "wkp": wkp_v,
            "wvp": wvp_v,
            "wlamp": wlamp_v,
            "wop": wop_v,
            "mstrip": mask,
            "selin": selv,
            "idin": idv,
            "onesin": ones2,
            "sel2in": sel2,
        })

    last_exc = None
    for attempt in range(3):
        try:
            res = run_bass_kernel_spmd(_get_nc(), in_maps, list(range(NC)),
                                       **_CACHE.get("run_kwargs", {}))
            break
        except Exception as e:  # transient NRT device wedges recover on retry
            last_exc = e
            _CACHE.pop("nc", None)
            import time as _time
            _time.sleep(5)
    else:
        raise last_exc
    _CACHE["last_res"] = res
    out = np.zeros((B, T, D), dtype=np.float32)
    for core in range(NC):
        out[core // 4] += res.results[core]["out"]
    return out



# revision 9
# speedup vs baseline: 1.4922x; 1.4922x over previous
"""DiffAttnV2-like fused kernel for Trainium2 (8 NeuronCores), v2.

Sharding: core = 4*b + g (b = batch 0..1, g = head-group 0..3, 4 output
heads each). Each core computes its 4 heads' attention and a partial
out = y_g @ Wo_g; host sums the 4 partials per batch.

v2 dataflow (all-bf16 matmuls; separate LDWEIGHTS hides weight loads):
  8 tq-blocks of 256. Per block: JIT projections (q/k transposed via
  W-stationary, v natural + lam via x-stationary) braided into the
  previous block's attention rounds; scores in [tk,(2 heads,256)] tiles
  with strip-matmul causal masking at 128 granularity; exp tiles become
  the *stationary* of the y matmul against a [v | ones] 129-col moving
  operand so the softmax denominator falls out as PSUM column 128;
  y lands natural [tq,d] so normalize/lambda-combine are per-partition
  DVE ops; PE transposes feed the Wo partial product.
"""
import sys
sys.path.insert(0, "/opt/trn_rl_repo")
from contextlib import ExitStack

import numpy as np
import ml_dtypes

from concourse import bacc, mybir, tile
from concourse.bass_utils import run_bass_kernel_spmd

B, T, D, H = 2, 2048, 2048, 16
NC = 8
NDC = D // 128        # 16 contraction chunks
NTB = 8               # tq blocks
TBW = 256             # tq block width
SCALE = 1.0 / float(np.sqrt(D // H))

f32 = mybir.dt.float32
bf16 = mybir.dt.bfloat16
EXP = mybir.ActivationFunctionType.Exp
SIG = mybir.ActivationFunctionType.Sigmoid
bfnp = ml_dtypes.bfloat16

_CACHE = {}


def _build():
    nc = bacc.Bacc("TRN2", target_bir_lowering=False, debug=False)
    xTp = nc.dram_tensor("xTp", [4, 128, NDC, 512], bf16, kind="ExternalInput").ap()
    wqkp = nc.dram_tensor("wqkp", [128, 12, NDC, 128], bf16, kind="ExternalInput").ap()
    wvlp = nc.dram_tensor("wvlp", [128, NDC, 516], bf16, kind="ExternalInput").ap()
    wop = nc.dram_tensor("wop", [128, 4, 4, 512], bf16, kind="ExternalInput").ap()
    stripin = nc.dram_tensor("stripin", [128, 384], bf16, kind="ExternalInput").ap()
    idin = nc.dram_tensor("idin", [128, 128], bf16, kind="ExternalInput").ap()
    out = nc.dram_tensor("out", [T, D], f32, kind="ExternalOutput").ap()

    with tile.TileContext(nc) as tc, ExitStack() as ctx:
        ctx.enter_context(nc.allow_low_precision(reason="bf16 matmul pipeline"))
        persist = ctx.enter_context(tc.tile_pool(name="persist", bufs=1))
        xpool = ctx.enter_context(tc.tile_pool(name="xpool", bufs=2))
        qpool = ctx.enter_context(tc.tile_pool(name="qpool", bufs=2))
        expool = ctx.enter_context(tc.tile_pool(name="expool", bufs=4))
        cpool = ctx.enter_context(tc.tile_pool(name="cpool", bufs=2))
        ycpool = ctx.enter_context(tc.tile_pool(name="ycpool", bufs=4))
        ytpool = ctx.enter_context(tc.tile_pool(name="ytpool", bufs=2))
        obpool = ctx.enter_context(tc.tile_pool(name="obpool", bufs=3))
        # PSUM: scores 2 + y 4 + misc 2 = 8 banks
        pbs = ctx.enter_context(tc.tile_pool(name="pbs", bufs=2, space="PSUM"))
        pby = ctx.enter_context(tc.tile_pool(name="pby", bufs=4, space="PSUM"))
        pbm = ctx.enter_context(tc.tile_pool(name="pbm", bufs=2, space="PSUM"))

        wqk = persist.tile([128, 12, NDC, 128], bf16)   # 48KB
        nc.sync.dma_start(out=wqk[:], in_=wqkp[:])
        wvl = persist.tile([128, NDC, 516], bf16)       # 16.1KB
        nc.sync.dma_start(out=wvl[:], in_=wvlp[:])
        wo = persist.tile([128, 4, 4, 512], bf16)       # 16KB
        nc.sync.dma_start(out=wo[:], in_=wop[:])
        strip = persist.tile([128, 384], bf16)
        nc.sync.dma_start(out=strip[:], in_=stripin[:])
        iden = persist.tile([128, 128], bf16)
        nc.sync.dma_start(out=iden[:], in_=idin[:])
        kT = persist.tile([128, 4, T], bf16)            # 16KB  [d,(kh,tk)]
        vn = persist.tile([128, 16, 4, 130], bf16)      # 16.6KB [tk,(tkc,kh,d+1)]
        nc.vector.memset(vn[:, :, :, 128:130], 1.0)     # ones col (+pad)
        lamS = persist.tile([128, 16, 4], f32)          # [t,(tchunk,hl)]

        xbt = [None] * 4
        qbt = [None] * 4

        def dma_x(pb):
            xbt[pb] = xpool.tile([128, NDC, 512], bf16, name=f"x{pb}", tag="x")
            nc.sync.dma_start(out=xbt[pb][:], in_=xTp[pb])

        def proj_chains(pb):
            """Thunks projecting pblock pb (512 tokens; needs xbt[pb])."""
            qbt[pb] = qpool.tile([128, 8, 512], bf16, name=f"q{pb}", tag="q")
            thunks = []

            def qk_chain(ch, pb=pb):
                def go():
                    ps = pbm.tile([128, 512], f32, name=f"pp{pb}_{ch}", tag="m")
                    for dc in range(NDC):
                        nc.tensor.matmul(ps[:], wqk[:, ch, dc], xbt[pb][:, dc],
                                         start=(dc == 0), stop=(dc == NDC - 1))
                    if ch < 8:
                        nc.vector.tensor_copy(qbt[pb][:, ch], ps[:])
                    else:
                        nc.vector.tensor_copy(
                            kT[:, ch - 8, 512 * pb:512 * (pb + 1)], ps[:])
                return go

            def vl_chain(ts, pb=pb):
                def go():
                    tchunk = 4 * pb + ts
                    psv = pbm.tile([128, 512], f32, name=f"pv{pb}_{ts}", tag="m")
                    for dc in range(NDC):
                        nc.tensor.matmul(
                            psv[:], xbt[pb][:, dc, 128 * ts:128 * (ts + 1)],
                            wvl[:, dc, 0:512], start=(dc == 0), stop=(dc == NDC - 1))
                    for kh in range(4):
                        nc.vector.tensor_copy(
                            vn[:, tchunk, kh, 0:128], psv[:, 128 * kh:128 * (kh + 1)])
                    psl = pbm.tile([128, 4], f32, name=f"pl{pb}_{ts}", tag="m")
                    for dc in range(NDC):
                        nc.tensor.matmul(
                            psl[:], xbt[pb][:, dc, 128 * ts:128 * (ts + 1)],
                            wvl[:, dc, 512:516], start=(dc == 0), stop=(dc == NDC - 1))
                    nc.scalar.activation(lamS[:, tchunk, :], psl[:], SIG)
                return go

            for ch in range(12):
                thunks.append(qk_chain(ch))
            for ts in range(4):
                thunks.append(vl_chain(ts))
            return thunks

        def attention(tb, braids):
            """Attention for block tb; pops braided proj thunks between rounds."""
            ntk = 2 * tb + 2
            pb, thalf = divmod(tb, 2)
            for hp in range(2):
                yt = {}
                for hlh in range(2):
                    for j in range(2):
                        yt[(hlh, j)] = pby.tile(
                            [128, 260], f32, name=f"y{tb}_{hp}_{hlh}_{j}", tag="y")
                pending = []
                for tkc in range(ntk):
                    for j in range(2):
                        khl = hp + 2 * j
                        ps_s = pbs.tile([128, 2, TBW], f32,
                                        name=f"s{tb}_{hp}_{tkc}_{j}", tag="s")
                        diag = tkc >= 2 * tb
                        qh = 2 * hp + 4 * j
                        nc.tensor.matmul(
                            ps_s.rearrange("p a b -> p (a b)"),
                            kT[:, khl, 128 * tkc:128 * (tkc + 1)],
                            qbt[pb][:, qh:qh + 2, TBW * thalf:TBW * (thalf + 1)],
                            start=True, stop=not diag)
                        if diag:
                            off = 128 if tkc == 2 * tb else 0
                            for hlh in range(2):
                                nc.tensor.matmul(
                                    ps_s[:, hlh], iden[:], strip[:, off:off + 256],
                                    start=False, stop=(hlh == 1))
                        ex = expool.tile([128, 2, TBW], bf16,
                                         name=f"e{tb}_{hp}_{tkc}_{j}", tag="ex")
                        nc.scalar.activation(ex[:], ps_s[:], EXP, scale=SCALE)
                        pending.append((j, khl, tkc, ex))
                    if len(pending) > 2:
                        for (j, khl, pk, ex) in pending[:2]:
                            for hlh in range(2):
                                for tqs in range(2):
                                    # start=True clears has_written for the
                                    # WHOLE bank: only the first sub-chain may
                                    # set it; tqs1's pk==0 write lands on
                                    # cleared bits and overwrites anyway.
                                    nc.tensor.matmul(
                                        yt[(hlh, j)][:, 130 * tqs:130 * tqs + 129],
                                        ex[:, hlh, 128 * tqs:128 * (tqs + 1)],
                                        vn[:, pk, khl, 0:129],
                                        start=(pk == 0 and tqs == 0),
                                        stop=(pk == ntk - 1),
                                        skip_group_check=True)
                        pending = pending[2:]
                    if braids:
                        braids.pop(0)()
                for (j, khl, pk, ex) in pending:
                    for hlh in range(2):
                        for tqs in range(2):
                            nc.tensor.matmul(
                                yt[(hlh, j)][:, 130 * tqs:130 * tqs + 129],
                                ex[:, hlh, 128 * tqs:128 * (tqs + 1)],
                                vn[:, pk, khl, 0:129],
                                start=(pk == 0 and tqs == 0),
                                stop=(pk == ntk - 1),
                                skip_group_check=True)
                # combine on DVE: yc = y0/den0 - lam*y1/den1  (natural [tq,d])
                for tqs in range(2):
                    tchunk = 2 * tb + tqs
                    for hlh in range(2):
                        hl = 2 * hp + hlh
                        y0, y1 = yt[(hlh, 0)], yt[(hlh, 1)]
                        rd = cpool.tile([128, 2], f32, name=f"rd{tb}_{hl}_{tqs}",
                                        tag="rd", bufs=4)
                        nc.vector.reciprocal_approx_fast(
                            rd[:, 0:1], y0[:, 130 * tqs + 128:130 * tqs + 129])
                        nc.vector.reciprocal_approx_fast(
                            rd[:, 1:2], y1[:, 130 * tqs + 128:130 * tqs + 129])
                        s1 = cpool.tile([128, 1], f32, name=f"s1{tb}_{hl}_{tqs}",
                                        tag="s1", bufs=4)
                        nc.vector.tensor_mul(s1[:], rd[:, 1:2],
                                             lamS[:, tchunk, hl:hl + 1])
                        t0 = cpool.tile([128, 128], f32, name=f"t0{tb}_{hl}_{tqs}",
                                        tag="t0", bufs=2)
                        nc.vector.tensor_scalar_mul(
                            t0[:], y0[:, 130 * tqs:130 * tqs + 128], rd[:, 0:1])
                        t1 = cpool.tile([128, 128], f32, name=f"t1{tb}_{hl}_{tqs}",
                                        tag="t1", bufs=2)
                        nc.vector.tensor_scalar_mul(
                            t1[:], y1[:, 130 * tqs:130 * tqs + 128], s1[:])
                        yc = ycs[tqs]
                        nc.vector.tensor_sub(yc[:, hl, :], t0[:], t1[:])

        for pb in range(4):
            if pb == 0:
                dma_x(0)
                for th in proj_chains(0):
                    th()
            if pb < 3:
                dma_x(pb + 1)
                braids = proj_chains(pb + 1)
            else:
                braids = []
            for thalf in range(2):
                tb = 2 * pb + thalf
                ycs = [ycpool.tile([128, 4, 128], bf16, name=f"yc{tb}_{t}",
                                   tag="yc") for t in range(2)]
                attention(tb, braids)
                if thalf == 1:
                    for th in braids:
                        th()
                    braids = []
                # transpose yc -> yT, then Wo partial
                yT = ytpool.tile([128, 4, 2, 128], bf16, name=f"yT{tb}", tag="yT")
                for tqs in range(2):
                    for hl in range(4):
                        pst = pbm.tile([128, 128], bf16, name=f"pt{tb}_{tqs}_{hl}",
                                       tag="m")
                        nc.tensor.transpose(pst[:], ycs[tqs][:, hl, :], iden[:])
                        nc.vector.tensor_copy(yT[:, hl, tqs, :], pst[:])
                for tqs in range(2):
                    for woc in range(4):
                        pso = pbm.tile([128, 512], f32, name=f"po{tb}_{tqs}_{woc}",
                                       tag="m")
                        for hl in range(4):
                            nc.tensor.matmul(pso[:], yT[:, hl, tqs], wo[:, hl, woc],
                                             start=(hl == 0), stop=(hl == 3))
                        ob = obpool.tile([128, 512], f32,
                                         name=f"ob{tb}_{tqs}_{woc}", tag="ob")
                        nc.vector.tensor_copy(ob[:], pso[:])
                        r0 = TBW * tb + 128 * tqs
                        nc.sync.dma_start(
                            out=out[r0:r0 + 128, 512 * woc:512 * (woc + 1)],
                            in_=ob[:])
    nc.compile()
    return nc


def _get_nc():
    if "nc" not in _CACHE:
        _CACHE["nc"] = _build()
    return _CACHE["nc"]


def kernel(x, Wq1, Wq2, Wk, Wv, Wlam, Wo, **_ignored):
    x = np.asarray(x, dtype=np.float32)
    Wq1 = np.asarray(Wq1, dtype=np.float32)
    Wq2 = np.asarray(Wq2, dtype=np.float32)
    Wk = np.asarray(Wk, dtype=np.float32)
    Wv = np.asarray(Wv, dtype=np.float32)
    Wlam = np.asarray(Wlam, dtype=np.float32)
    Wo = np.asarray(Wo, dtype=np.float32)

    rr = np.arange(128)[:, None]
    xx = np.arange(384)[None, :]
    strip = np.where(xx >= rr + 128, 0.0, -1e30).astype(bfnp)
    idv = np.eye(128, dtype=np.float32).astype(bfnp)

    xTs = []
    for b in range(B):
        xt = x[b].T.astype(bfnp)
        xTs.append(np.ascontiguousarray(
            xt.reshape(NDC, 128, 4, 512).transpose(2, 1, 0, 3)))

    in_maps = []
    for core in range(NC):
        b, g = divmod(core, 4)
        kv_cols = np.r_[256 * g:256 * g + 256, 1024 + 256 * g:1024 + 256 * g + 256]
        wqk = np.concatenate([Wq1[:, 512 * g:512 * (g + 1)],
                              Wq2[:, 512 * g:512 * (g + 1)],
                              Wk[:, kv_cols]], axis=1).astype(bfnp)  # [D,1536]
        wqkp_v = np.ascontiguousarray(
            wqk.reshape(NDC, 128, 12, 128).transpose(1, 2, 0, 3))
        wvl = np.concatenate([Wv[:, kv_cols], Wlam[:, 4 * g:4 * (g + 1)]],
                             axis=1).astype(bfnp)                    # [D,516]
        wvlp_v = np.ascontiguousarray(
            wvl.reshape(NDC, 128, 516).transpose(1, 0, 2))
        wo_s = Wo[512 * g:512 * (g + 1), :].astype(bfnp)             # [512,D]
        wop_v = np.ascontiguousarray(
            wo_s.reshape(4, 128, 4, 512).transpose(1, 0, 2, 3))
        in_maps.append({
            "xTp": xTs[b],
            "wqkp": wqkp_v,
            "wvlp": wvlp_v,
            "wop": wop_v,
            "stripin": strip,
            "idin": idv,
        })

    last_exc = None
    for attempt in range(3):
        try:
            res = run_bass_kernel_spmd(_get_nc(), in_maps, list(range(NC)),
                                       **_CACHE.get("run_kwargs", {}))
            break
        except Exception as e:  # transient NRT device wedges recover on retry
            last_exc = e
            _CACHE.pop("nc", None)
            import time as _time
            _time.sleep(5)
    else:
        raise last_exc
    _CACHE["last_res"] = res
    out = np.zeros((B, T, D), dtype=np.float32)
    for core in range(NC):
        out[core // 4] += res.results[core]["out"]
    return out


# revision 17
# speedup vs baseline: 1.5853x; 1.0624x over previous
"""DiffAttnV2-like fused kernel for Trainium2 (8 NeuronCores), v2.

Sharding: core = 4*b + g (b = batch 0..1, g = head-group 0..3, 4 output
heads each). Each core computes its 4 heads' attention and a partial
out = y_g @ Wo_g; host sums the 4 partials per batch.

v2 dataflow (all-bf16 matmuls; separate LDWEIGHTS hides weight loads):
  8 tq-blocks of 256. Per block: JIT projections (q/k transposed via
  W-stationary, v natural + lam via x-stationary) braided into the
  previous block's attention rounds; scores in [tk,(2 heads,256)] tiles
  with strip-matmul causal masking at 128 granularity; exp tiles become
  the *stationary* of the y matmul against a [v | ones] 129-col moving
  operand so the softmax denominator falls out as PSUM column 128;
  y lands natural [tq,d] so normalize/lambda-combine are per-partition
  DVE ops; PE transposes feed the Wo partial product.
"""
import sys
sys.path.insert(0, "/opt/trn_rl_repo")
from contextlib import ExitStack

import numpy as np
import ml_dtypes

from concourse import bacc, mybir, tile
from concourse.bass_utils import run_bass_kernel_spmd

B, T, D, H = 2, 2048, 2048, 16
NC = 8
NDC = D // 128        # 16 contraction chunks
NTB = 8               # tq blocks
TBW = 256             # tq block width
SCALE = 1.0 / float(np.sqrt(D // H))

f32 = mybir.dt.float32
bf16 = mybir.dt.bfloat16
EXP = mybir.ActivationFunctionType.Exp
SIG = mybir.ActivationFunctionType.Sigmoid
bfnp = ml_dtypes.bfloat16

_CACHE = {}


def _build():
    nc = bacc.Bacc("TRN2", target_bir_lowering=False, debug=False)
    xTp = nc.dram_tensor("xTp", [4, 128, NDC, 512], bf16, kind="ExternalInput").ap()
    wqkp = nc.dram_tensor("wqkp", [12, 128, NDC, 128], bf16, kind="ExternalInput").ap()
    wvlp = nc.dram_tensor("wvlp", [128, NDC, 516], bf16, kind="ExternalInput").ap()
    wop = nc.dram_tensor("wop", [128, 4, 4, 512], bf16, kind="ExternalInput").ap()
    stripin = nc.dram_tensor("stripin", [128, 384], bf16, kind="ExternalInput").ap()
    idin = nc.dram_tensor("idin", [128, 128], bf16, kind="ExternalInput").ap()
    out = nc.dram_tensor("out", [T, D], f32, kind="ExternalOutput").ap()

    with tile.TileContext(nc) as tc, ExitStack() as ctx:
        ctx.enter_context(nc.allow_low_precision(reason="bf16 matmul pipeline"))
        persist = ctx.enter_context(tc.tile_pool(name="persist", bufs=1))
        xpool = ctx.enter_context(tc.tile_pool(name="xpool", bufs=2))
        qpool = ctx.enter_context(tc.tile_pool(name="qpool", bufs=2))
        expool = ctx.enter_context(tc.tile_pool(name="expool", bufs=4))
        cpool = ctx.enter_context(tc.tile_pool(name="cpool", bufs=2))
        ycpool = ctx.enter_context(tc.tile_pool(name="ycpool", bufs=4))
        ytpool = ctx.enter_context(tc.tile_pool(name="ytpool", bufs=2))
        obpool = ctx.enter_context(tc.tile_pool(name="obpool", bufs=3))
        # PSUM: scores 2 + y 4 + misc 2 = 8 banks
        pbs = ctx.enter_context(tc.tile_pool(name="pbs", bufs=2, space="PSUM"))
        pby = ctx.enter_context(tc.tile_pool(name="pby", bufs=4, space="PSUM"))
        pbm = ctx.enter_context(tc.tile_pool(name="pbm", bufs=2, space="PSUM"))

        wqk = persist.tile([128, 12, NDC, 128], bf16)   # 48KB
        wvl = persist.tile([128, NDC, 516], bf16)       # 16.1KB
        wo = persist.tile([128, 4, 4, 512], bf16)       # 16KB
        strip = persist.tile([128, 384], bf16)          # 0/1 causal strip
        iden = persist.tile([128, 128], bf16)
        kT = persist.tile([128, 4, T], bf16)            # 16KB  [d,(kh,tk)]
        vn = persist.tile([128, 16, 4, 130], bf16)      # 16.6KB [tk,(tkc,kh,d+1)]
        nc.vector.memset(vn[:, :, :, 128:130], 1.0)     # ones col (+pad)
        lamS = persist.tile([128, 16, 4], f32)          # [t,(tchunk,hl)]

        xbt = [None] * 4
        qbt = [None] * 4

        def dma_x(pb):
            xbt[pb] = xpool.tile([128, NDC, 512], bf16, name=f"x{pb}", tag="x")
            nc.sync.dma_start(out=xbt[pb][:], in_=xTp[pb])

        def proj_chains(pb):
            """Thunks projecting pblock pb (512 tokens; needs xbt[pb])."""
            qbt[pb] = qpool.tile([128, 8, 512], bf16, name=f"q{pb}", tag="q")
            thunks = []

            def qk_chain(ch, pb=pb):
                def go():
                    ps = pbm.tile([128, 512], f32, name=f"pp{pb}_{ch}", tag="m")
                    for dc in range(NDC):
                        nc.tensor.matmul(ps[:], wqk[:, ch, dc], xbt[pb][:, dc],
                                         start=(dc == 0), stop=(dc == NDC - 1))
                    if ch < 8:
                        nc.vector.tensor_copy(qbt[pb][:, ch], ps[:])
                    else:
                        nc.vector.tensor_copy(
                            kT[:, ch - 8, 512 * pb:512 * (pb + 1)], ps[:])
                return go

            def vl_chain(ts, pb=pb):
                def go():
                    tchunk = 4 * pb + ts
                    psv = pbm.tile([128, 512], f32, name=f"pv{pb}_{ts}", tag="m")
                    for dc in range(NDC):
                        nc.tensor.matmul(
                            psv[:], xbt[pb][:, dc, 128 * ts:128 * (ts + 1)],
                            wvl[:, dc, 0:512], start=(dc == 0), stop=(dc == NDC - 1))
                    for kh in range(4):
                        nc.vector.tensor_copy(
                            vn[:, tchunk, kh, 0:128], psv[:, 128 * kh:128 * (kh + 1)])
                    psl = pbm.tile([128, 4], f32, name=f"pl{pb}_{ts}", tag="m")
                    for dc in range(NDC):
                        nc.tensor.matmul(
                            psl[:], xbt[pb][:, dc, 128 * ts:128 * (ts + 1)],
                            wvl[:, dc, 512:516], start=(dc == 0), stop=(dc == NDC - 1))
                    # sigmoid via the Exp table (avoids ACT table reloads):
                    # lam = 1 / (1 + exp(-z))
                    el = cpool.tile([128, 4], f32, name=f"el{pb}_{ts}", tag="el",
                                    bufs=2)
                    nc.scalar.activation(el[:], psl[:], EXP, scale=-1.0)
                    nc.vector.tensor_scalar_add(el[:], el[:], 1.0)
                    nc.vector.reciprocal_approx_fast(lamS[:, tchunk, :], el[:])
                return go

            for ch in range(12):
                thunks.append(qk_chain(ch))
            for ts in range(4):
                thunks.append(vl_chain(ts))
            return thunks

        def attention(tb, braids):
            """Attention for block tb; pops braided proj thunks between rounds."""
            ntk = 2 * tb + 2
            pb, thalf = divmod(tb, 2)
            for hp in range(2):
                yt = {}
                for hlh in range(2):
                    for j in range(2):
                        yt[(hlh, j)] = pby.tile(
                            [128, 260], f32, name=f"y{tb}_{hp}_{hlh}_{j}", tag="y")
                pending = []
                for tkc in range(ntk):
                    for j in range(2):
                        khl = hp + 2 * j
                        ps_s = pbs.tile([128, 2, TBW], f32,
                                        name=f"s{tb}_{hp}_{tkc}_{j}", tag="s")
                        diag = tkc >= 2 * tb
                        qh = 2 * hp + 4 * j
                        nc.tensor.matmul(
                            ps_s.rearrange("p a b -> p (a b)"),
                            kT[:, khl, 128 * tkc:128 * (tkc + 1)],
                            qbt[pb][:, qh:qh + 2, TBW * thalf:TBW * (thalf + 1)],
                            start=True, stop=True)
                        ex = expool.tile([128, 2, TBW], bf16,
                                         name=f"e{tb}_{hp}_{tkc}_{j}", tag="ex")
                        nc.scalar.activation(ex[:], ps_s[:], EXP, scale=SCALE)
                        if diag:
                            # zero the upper triangle on DVE (0/1 strip)
                            off = 128 if tkc == 2 * tb else 0
                            for hlh in range(2):
                                nc.vector.tensor_mul(
                                    ex[:, hlh], ex[:, hlh], strip[:, off:off + 256])
                        pending.append((j, khl, tkc, ex))
                    if len(pending) > 2:
                        for (j, khl, pk, ex) in pending[:2]:
                            for hlh in range(2):
                                for tqs in range(2):
                                    # start=True clears has_written for the
                                    # WHOLE bank: only the first sub-chain may
                                    # set it; tqs1's pk==0 write lands on
                                    # cleared bits and overwrites anyway.
                                    nc.tensor.matmul(
                                        yt[(hlh, j)][:, 130 * tqs:130 * tqs + 129],
                                        ex[:, hlh, 128 * tqs:128 * (tqs + 1)],
                                        vn[:, pk, khl, 0:129],
                                        start=(pk == 0 and tqs == 0),
                                        stop=(pk == ntk - 1),
                                        skip_group_check=True)
                        pending = pending[2:]
                    if braids:
                        braids.pop(0)()
                for (j, khl, pk, ex) in pending:
                    for hlh in range(2):
                        for tqs in range(2):
                            nc.tensor.matmul(
                                yt[(hlh, j)][:, 130 * tqs:130 * tqs + 129],
                                ex[:, hlh, 128 * tqs:128 * (tqs + 1)],
                                vn[:, pk, khl, 0:129],
                                start=(pk == 0 and tqs == 0),
                                stop=(pk == ntk - 1),
                                skip_group_check=True)
                # combine on DVE: yc = y0/den0 - lam*y1/den1  (natural [tq,d])
                for tqs in range(2):
                    tchunk = 2 * tb + tqs
                    for hlh in range(2):
                        hl = 2 * hp + hlh
                        y0, y1 = yt[(hlh, 0)], yt[(hlh, 1)]
                        rd = cpool.tile([128, 2], f32, name=f"rd{tb}_{hl}_{tqs}",
                                        tag="rd", bufs=4)
                        nc.vector.reciprocal_approx_fast(
                            rd[:, 0:1], y0[:, 130 * tqs + 128:130 * tqs + 129])
                        nc.vector.reciprocal_approx_fast(
                            rd[:, 1:2], y1[:, 130 * tqs + 128:130 * tqs + 129])
                        s1 = cpool.tile([128, 1], f32, name=f"s1{tb}_{hl}_{tqs}",
                                        tag="s1", bufs=4)
                        nc.vector.tensor_mul(s1[:], rd[:, 1:2],
                                             lamS[:, tchunk, hl:hl + 1])
                        t0 = cpool.tile([128, 128], f32, name=f"t0{tb}_{hl}_{tqs}",
                                        tag="t0", bufs=2)
                        nc.vector.tensor_scalar_mul(
                            t0[:], y0[:, 130 * tqs:130 * tqs + 128], rd[:, 0:1])
                        t1 = cpool.tile([128, 128], f32, name=f"t1{tb}_{hl}_{tqs}",
                                        tag="t1", bufs=2)
                        nc.vector.tensor_scalar_mul(
                            t1[:], y1[:, 130 * tqs:130 * tqs + 128], s1[:])
                        yc = ycs[tqs]
                        nc.vector.tensor_sub(yc[:, hl, :], t0[:], t1[:])

        for pb in range(4):
            if pb == 0:
                dma_x(0)
                for ch in range(12):
                    nc.sync.dma_start(out=wqk[:, ch], in_=wqkp[ch])
                nc.sync.dma_start(out=wvl[:], in_=wvlp[:])
                nc.sync.dma_start(out=strip[:], in_=stripin[:])
                nc.sync.dma_start(out=wo[:], in_=wop[:])
                nc.sync.dma_start(out=iden[:], in_=idin[:])
                for th in proj_chains(0):
                    th()
            if pb < 3:
                dma_x(pb + 1)
                braids = proj_chains(pb + 1)
            else:
                braids = []
            for thalf in range(2):
                tb = 2 * pb + thalf
                ycs = [ycpool.tile([128, 4, 128], bf16, name=f"yc{tb}_{t}",
                                   tag="yc") for t in range(2)]
                attention(tb, braids)
                if thalf == 1:
                    for th in braids:
                        th()
                    braids = []
                # transpose yc -> yT, then Wo partial
                yT = ytpool.tile([128, 4, 2, 128], bf16, name=f"yT{tb}", tag="yT")
                for tqs in range(2):
                    for hl in range(4):
                        pst = pbm.tile([128, 128], bf16, name=f"pt{tb}_{tqs}_{hl}",
                                       tag="m")
                        nc.tensor.transpose(pst[:], ycs[tqs][:, hl, :], iden[:])
                        nc.vector.tensor_copy(yT[:, hl, tqs, :], pst[:])
                for tqs in range(2):
                    for woc in range(4):
                        pso = pbm.tile([128, 512], f32, name=f"po{tb}_{tqs}_{woc}",
                                       tag="m")
                        for hl in range(4):
                            nc.tensor.matmul(pso[:], yT[:, hl, tqs], wo[:, hl, woc],
                                             start=(hl == 0), stop=(hl == 3))
                        ob = obpool.tile([128, 512], f32,
                                         name=f"ob{tb}_{tqs}_{woc}", tag="ob")
                        nc.vector.tensor_copy(ob[:], pso[:])
                        r0 = TBW * tb + 128 * tqs
                        nc.sync.dma_start(
                            out=out[r0:r0 + 128, 512 * woc:512 * (woc + 1)],
                            in_=ob[:])
    nc.compile()
    return nc


def _get_nc():
    if "nc" not in _CACHE:
        _CACHE["nc"] = _build()
    return _CACHE["nc"]


def kernel(x, Wq1, Wq2, Wk, Wv, Wlam, Wo, **_ignored):
    x = np.asarray(x, dtype=np.float32)
    Wq1 = np.asarray(Wq1, dtype=np.float32)
    Wq2 = np.asarray(Wq2, dtype=np.float32)
    Wk = np.asarray(Wk, dtype=np.float32)
    Wv = np.asarray(Wv, dtype=np.float32)
    Wlam = np.asarray(Wlam, dtype=np.float32)
    Wo = np.asarray(Wo, dtype=np.float32)

    rr = np.arange(128)[:, None]
    xx = np.arange(384)[None, :]
    strip = np.where(xx >= rr + 128, 1.0, 0.0).astype(bfnp)
    idv = np.eye(128, dtype=np.float32).astype(bfnp)

    xTs = []
    for b in range(B):
        xt = x[b].T.astype(bfnp)
        xTs.append(np.ascontiguousarray(
            xt.reshape(NDC, 128, 4, 512).transpose(2, 1, 0, 3)))

    in_maps = []
    for core in range(NC):
        b, g = divmod(core, 4)
        kv_cols = np.r_[256 * g:256 * g + 256, 1024 + 256 * g:1024 + 256 * g + 256]
        wqk = np.concatenate([Wq1[:, 512 * g:512 * (g + 1)],
                              Wq2[:, 512 * g:512 * (g + 1)],
                              Wk[:, kv_cols]], axis=1).astype(bfnp)  # [D,1536]
        wqkp_v = np.ascontiguousarray(
            wqk.reshape(NDC, 128, 12, 128).transpose(2, 1, 0, 3))
        wvl = np.concatenate([Wv[:, kv_cols], Wlam[:, 4 * g:4 * (g + 1)]],
                             axis=1).astype(bfnp)                    # [D,516]
        wvlp_v = np.ascontiguousarray(
            wvl.reshape(NDC, 128, 516).transpose(1, 0, 2))
        wo_s = Wo[512 * g:512 * (g + 1), :].astype(bfnp)             # [512,D]
        wop_v = np.ascontiguousarray(
            wo_s.reshape(4, 128, 4, 512).transpose(1, 0, 2, 3))
        in_maps.append({
            "xTp": xTs[b],
            "wqkp": wqkp_v,
            "wvlp": wvlp_v,
            "wop": wop_v,
            "stripin": strip,
            "idin": idv,
        })

    last_exc = None
    for attempt in range(3):
        try:
            res = run_bass_kernel_spmd(_get_nc(), in_maps, list(range(NC)),
                                       **_CACHE.get("run_kwargs", {}))
            break
        except Exception as e:  # transient NRT device wedges recover on retry
            last_exc = e
            _CACHE.pop("nc", None)
            import time as _time
            _time.sleep(5)
    else:
        raise last_exc
    _CACHE["last_res"] = res
    out = np.zeros((B, T, D), dtype=np.float32)
    for core in range(NC):
        out[core // 4] += res.results[core]["out"]
    return out
